# revision 16
# baseline (speedup 1.0000x reference)
"""Trainium2 Bass kernel for nn_AttachmentPredictor.

Computation (per batch row b):
  head = x[b, :-2, :] @ proj_head + x[b,-2,:] @ proj_prep + x[b,-1,:] @ proj_child
  composed = tanh(head)                      # [T-2, P]
  composed = tanh(composed @ hidden_W[0])
  composed = tanh(composed @ hidden_W[1])
  scores = composed @ scorer                 # [T-2]
  out = where(mask, exp(scores), 0); out /= (sum(out) + 1e-7)

Sharding: pure data parallel, batch 64 -> 8 rows per core on 8 cores.

On-chip layout: activations kept transposed [P on partitions, tokens on free
dim].  x tiles are loaded naturally [tok, d] and transposed on the tensor
engine ([128,128] blocks via identity matmul).  All matmuls use float32r
(full-rate fp32 streaming).  The 2046 head tokens per row are processed as
2048 (the prep/child rows ride along as garbage and are masked out).
"""

import sys

import numpy as np

sys.path.insert(0, "/opt/trn_rl_repo")

B = 64
T = 2048
TH = 2046  # head tokens
D = 1024
P = 512
NCORES = 8
R = B // NCORES  # 8 batch rows per core
KD = D // 128  # 8 contraction chunks for layer 1
KP = P // 128  # 4 contraction chunks for layers 2/3/scorer
NTOK = 512  # tokens per chunk
CH = T // NTOK  # 4 chunks per row
J16 = T // 128  # 16 score sub-chunks of 128 tokens per row

X_BF16 = False  # if True: ship x and layer-1 weights as bf16
SAFE_SCORER = False  # if True: scorer matmuls write bank-offset-0 PSUM tiles
_CACHE = {}


def _build(R=R):
    import concourse.bass as bass
    import concourse.mybir as mybir
    import concourse.tile as tile
    from concourse import bacc
    from concourse.masks import make_identity

    f32 = mybir.dt.float32
    f32r = mybir.dt.float32r
    bf16 = mybir.dt.bfloat16
    u8 = mybir.dt.uint8
    xdt = bf16 if X_BF16 else f32r
    bdt = bf16 if X_BF16 else f32
    AF = mybir.ActivationFunctionType
    ALU = mybir.AluOpType

    nc = bacc.Bacc(
        "TRN2", target_bir_lowering=False, debug=False, num_devices=NCORES
    )

    xs = nc.dram_tensor("xs", [R, T, D], xdt, kind="ExternalInput").ap()
    w1 = nc.dram_tensor("w1", [D, P], xdt, kind="ExternalInput").ap()
    wp = nc.dram_tensor("wp", [D, P], bdt, kind="ExternalInput").ap()
    wc = nc.dram_tensor("wc", [D, P], bdt, kind="ExternalInput").ap()
    h0 = nc.dram_tensor("h0", [P, P], f32r, kind="ExternalInput").ap()
    h1 = nc.dram_tensor("h1", [P, P], f32r, kind="ExternalInput").ap()
    sc = nc.dram_tensor("sc", [P, 1], f32, kind="ExternalInput").ap()
    mk = nc.dram_tensor("mk", [R, T], u8, kind="ExternalInput").ap()
    out = nc.dram_tensor("out", [R, TH], f32, kind="ExternalOutput").ap()

    with tile.TileContext(nc) as tc:
        with (
            tc.tile_pool(name="wpool", bufs=1) as wpool,
            tc.tile_pool(name="cpool", bufs=1) as cpool,
            tc.tile_pool(name="xn_pool", bufs=2) as xn_pool,
            tc.tile_pool(name="xt_pool", bufs=2 * KD) as xt_pool,
            tc.tile_pool(name="y_pool", bufs=2 * KP) as y_pool,
            tc.tile_pool(name="tail_pool", bufs=2) as tail_pool,
            tc.tile_pool(name="xtp_pool", bufs=2, space="PSUM") as xtp_pool,
            tc.tile_pool(name="mmp_pool", bufs=3, space="PSUM") as mmp_pool,
            tc.tile_pool(name="scp_pool", bufs=1, space="PSUM") as scp_pool,
            tc.tile_pool(name="tlp_pool", bufs=2, space="PSUM") as tlp_pool,
        ):
            # ---- weights: [p, k, q] = W[k*128 + p, q] ----
            w1t = wpool.tile([128, KD, P], xdt)
            wpt = wpool.tile([128, KD, P], bdt)
            wct = wpool.tile([128, KD, P], bdt)
            for k in range(KD):
                nc.sync.dma_start(w1t[:, k, :], w1[k * 128 : (k + 1) * 128, :])
                nc.sync.dma_start(wpt[:, k, :], wp[k * 128 : (k + 1) * 128, :])
                nc.sync.dma_start(wct[:, k, :], wc[k * 128 : (k + 1) * 128, :])
            h0t = wpool.tile([128, KP, P], f32r)
            h1t = wpool.tile([128, KP, P], f32r)
            sct = wpool.tile([128, KP], f32)
            for k in range(KP):
                nc.sync.dma_start(h0t[:, k, :], h0[k * 128 : (k + 1) * 128, :])
                nc.sync.dma_start(h1t[:, k, :], h1[k * 128 : (k + 1) * 128, :])
                nc.sync.dma_start(sct[:, k : k + 1], sc[k * 128 : (k + 1) * 128, :])

            ident_f = cpool.tile([128, 128], f32)
            make_identity(nc, ident_f[:])
            ident_r = cpool.tile([128, 128], xdt)
            nc.vector.tensor_copy(ident_r[:], ident_f[:])
            ones128x16 = cpool.tile([128, 16], f32)
            nc.vector.memset(ones128x16[:], 1.0)
            rs128 = cpool.tile([128, 1], f32)
            nc.vector.memset(rs128[:], 0.0)

            # ---- per-row bias: biasT[p, m, r] = (prep_r @ wp + child_r @ wc)[m*128+p]
            pc_prep = cpool.tile([128, KD, R], bdt)
            pc_child = cpool.tile([128, KD, R], bdt)
            for r in range(R):
                for k in range(KD):
                    src_p = xs[r, T - 2, k * 128 : (k + 1) * 128].unsqueeze(-1)
                    src_c = xs[r, T - 1, k * 128 : (k + 1) * 128].unsqueeze(-1)
                    if not X_BF16:
                        src_p = src_p.bitcast(bdt)
                        src_c = src_c.bitcast(bdt)
                    nc.sync.dma_start(pc_prep[:, k, r : r + 1], src_p)
                    nc.sync.dma_start(pc_child[:, k, r : r + 1], src_c)
            biasT = cpool.tile([128, KP, R], f32)
            for m in range(KP):
                bps = mmp_pool.tile([128, R], f32, tag="mm")
                for k in range(KD):
                    nc.tensor.matmul(
                        bps[:],
                        wpt[:, k, m * 128 : (m + 1) * 128],
                        pc_prep[:, k, :],
                        start=(k == 0),
                        stop=False,
                    )
                for k in range(KD):
                    nc.tensor.matmul(
                        bps[:],
                        wct[:, k, m * 128 : (m + 1) * 128],
                        pc_child[:, k, :],
                        start=False,
                        stop=(k == KD - 1),
                    )
                nc.vector.tensor_copy(biasT[:, m, :], bps[:])

            # ---- main loop ----
            for r in range(R):
                if SAFE_SCORER:
                    s_sb = tail_pool.tile([128, J16], f32, tag="ssb")
                    sc_ps = None
                else:
                    sc_ps = scp_pool.tile([128, J16], f32, tag="scps")
                for c in range(CH):
                    xn = xn_pool.tile([128, 4, D], xdt, tag="xn")
                    for jj in range(4):
                        t0 = c * NTOK + jj * 128
                        nc.sync.dma_start(xn[:, jj, :], xs[r, t0 : t0 + 128, :])
                    # transpose x to [d, tok]
                    xts = []
                    for k in range(KD):
                        xp = xtp_pool.tile([128, NTOK], xdt, tag="xtps")
                        for jj in range(4):
                            nc.tensor.transpose(
                                xp[:, jj * 128 : (jj + 1) * 128],
                                xn[:, jj, k * 128 : (k + 1) * 128],
                                ident_r[:],
                            )
                        xt = xt_pool.tile([128, NTOK], xdt, tag="xt")
                        nc.vector.tensor_copy(xt[:], xp[:])
                        xts.append(xt)
                    # layer 1: y1 = tanh(W1.T @ xT + bias)
                    y1s = []
                    for m in range(KP):
                        ps = mmp_pool.tile([128, NTOK], f32, tag="mm")
                        for k in range(KD):
                            nc.tensor.matmul(
                                ps[:],
                                w1t[:, k, m * 128 : (m + 1) * 128],
                                xts[k][:],
                                start=(k == 0),
                                stop=(k == KD - 1),
                            )
                        y1 = y_pool.tile([128, NTOK], f32r, tag="y1")
                        nc.scalar.activation(
                            y1[:], ps[:], AF.Tanh, bias=biasT[:, m, r : r + 1]
                        )
                        y1s.append(y1)
                    # layer 2
                    y2s = []
                    for m in range(KP):
                        ps = mmp_pool.tile([128, NTOK], f32, tag="mm")
                        for k in range(KP):
                            nc.tensor.matmul(
                                ps[:],
                                h0t[:, k, m * 128 : (m + 1) * 128],
                                y1s[k][:],
                                start=(k == 0),
                                stop=(k == KP - 1),
                            )
                        y2 = y_pool.tile([128, NTOK], f32r, tag="y2")
                        nc.scalar.activation(y2[:], ps[:], AF.Tanh)
                        y2s.append(y2)
                    # layer 3
                    y3s = []
                    for m in range(KP):
                        ps = mmp_pool.tile([128, NTOK], f32, tag="mm")
                        for k in range(KP):
                            nc.tensor.matmul(
                                ps[:],
                                h1t[:, k, m * 128 : (m + 1) * 128],
                                y2s[k][:],
                                start=(k == 0),
                                stop=(k == KP - 1),
                            )
                        y3 = y_pool.tile([128, NTOK], f32, tag="y3")
                        nc.scalar.activation(y3[:], ps[:], AF.Tanh)
                        y3s.append(y3)
                    # scorer: scores land [tok-on-partitions]
                    for jj in range(4):
                        col = c * 4 + jj
                        if SAFE_SCORER:
                            s1 = mmp_pool.tile([128, 1], f32, tag="mm")
                            for k in range(KP):
                                nc.tensor.matmul(
                                    s1[:],
                                    y3s[k][:, jj * 128 : (jj + 1) * 128],
                                    sct[:, k : k + 1],
                                    start=(k == 0),
                                    stop=(k == KP - 1),
                                )
                            nc.vector.tensor_copy(s_sb[:, col : col + 1], s1[:])
                        else:
                            for k in range(KP):
                                nc.tensor.matmul(
                                    sc_ps[:, col : col + 1],
                                    y3s[k][:, jj * 128 : (jj + 1) * 128],
                                    sct[:, k : k + 1],
                                    start=(k == 0),
                                    stop=(k == KP - 1),
                                )
                # ---- tail: masked softmax over the row ----
                # exp into cols 0:16 of a 128-wide pad tile; full-width PE
                # transpose; only rows 0:16 of the result are read.
                e_pad = tail_pool.tile([128, 128], f32, tag="esb")
                nc.scalar.activation(e_pad[:, 0:J16], s_sb[:] if SAFE_SCORER else sc_ps[:], AF.Exp)
                et_ps = tlp_pool.tile([128, 128], f32, tag="tl")
                nc.tensor.transpose(et_ps[:], e_pad[:], ident_f[:])
                mku8 = tail_pool.tile([16, 128], u8, tag="mku8")
                nc.sync.dma_start(
                    mku8[:], mk[r, 0:2048].rearrange("(j p) -> j p", p=128)
                )
                mf = tail_pool.tile([16, 128], f32, tag="mf")
                nc.vector.tensor_copy(mf[:], mku8[:])
                me = tail_pool.tile([16, 128], f32, tag="me")
                nc.vector.tensor_tensor(
                    out=me[:], in0=et_ps[0:16, :], in1=mf[:], op=ALU.mult
                )
                rs = tail_pool.tile([16, 1], f32, tag="rs")
                nc.vector.reduce_sum(rs[:], me[:], axis=mybir.AxisListType.X)
                nc.vector.tensor_copy(rs128[0:16, :], rs[:])
                rb_ps = tlp_pool.tile([16, 1], f32, tag="tl")
                nc.tensor.matmul(rb_ps[:], ones128x16[:], rs128[:])
                rb = tail_pool.tile([16, 1], f32, tag="rb")
                nc.vector.tensor_scalar_add(rb[:], rb_ps[:], 1e-7)
                rcp = tail_pool.tile([16, 1], f32, tag="rcp")
                nc.vector.reciprocal(rcp[:], rb[:])
                ot = tail_pool.tile([16, 128], f32, tag="ot")
                nc.vector.tensor_scalar_mul(ot[:], me[:], rcp[:])
                nc.sync.dma_start(
                    out[r, 0:1920].rearrange("(j p) -> j p", p=128), ot[0:15, :]
                )
                nc.sync.dma_start(
                    out[r, 1920:2046].rearrange("(j p) -> j p", p=126),
                    ot[15:16, 0:126],
                )
    nc.compile()
    return nc


def _get_nc():
    if "nc" not in _CACHE:
        _CACHE["nc"] = _build()
    return _CACHE["nc"]


def _make_in_maps(inputs):
    import ml_dtypes

    xdt = ml_dtypes.bfloat16 if X_BF16 else np.float32
    x = np.ascontiguousarray(np.asarray(inputs["x"], dtype=np.float32).astype(xdt))
    w1 = np.ascontiguousarray(np.asarray(inputs["proj_head"], dtype=np.float32).astype(xdt))
    wp = np.ascontiguousarray(np.asarray(inputs["proj_prep"], dtype=np.float32).astype(xdt))
    wc = np.ascontiguousarray(np.asarray(inputs["proj_child"], dtype=np.float32).astype(xdt))
    hw = np.asarray(inputs["hidden_W"], dtype=np.float32)
    sc = np.ascontiguousarray(np.asarray(inputs["scorer"], dtype=np.float32))
    mk = np.asarray(inputs["mask"]).astype(np.uint8).copy()
    mk[:, TH:] = 0  # prep/child rows are never head candidates
    in_maps = []
    for i in range(NCORES):
        in_maps.append(
            {
                "xs": np.ascontiguousarray(x[i * R : (i + 1) * R]),
                "w1": w1,
                "wp": wp,
                "wc": wc,
                "h0": np.ascontiguousarray(hw[0]),
                "h1": np.ascontiguousarray(hw[1]),
                "sc": sc,
                "mk": np.ascontiguousarray(mk[i * R : (i + 1) * R]),
            }
        )
    return in_maps


def _run(inputs, **kwargs):
    from concourse.bass_utils import run_bass_kernel_spmd

    nc = _get_nc()
    res = run_bass_kernel_spmd(
        nc, _make_in_maps(inputs), core_ids=list(range(NCORES)), **kwargs
    )
    out = np.concatenate([res.results[i]["out"] for i in range(NCORES)], axis=0)
    return out, res


def kernel(**inputs) -> np.ndarray:
    out, _ = _run(inputs)
    return out
